# revision 13
# baseline (speedup 1.0000x reference)
"""Relational GAT message-passing kernel for 8 Trainium2 NeuronCores.

Strategy (zero-collective, 1D row partitioning):
  - Edges are sharded by subject-node range: core c owns all edges whose
    edge_sub falls in [c*N/8, (c+1)*N/8). Segment rows (sub + pred*N) for
    those subjects live entirely on that core, so segment softmax stats and
    the scatter-add need no cross-core reduction at all.
  - Within a core, edges are grouped into "windows" = (pred, 128-subject
    block). Each window's edges are padded to TPW tiles of 128 edge slots.
  - The only per-edge DRAM traffic is one indirect-DMA gather of x[obj]
    per 128-edge tile. Everything else is dense matmul/vector work:
      dot[e,h] = sum_j x[obj_e,(h,j)] * kq[sub_e,(h,j)], where
      kq = (x_window @ Wk_bd) @ Wq_bd  is computed once per window, and
      the per-edge selection kq[sub_e] is a one-hot selector matmul.
      The selector G is built on-chip from the edge row-ids with an
      iota compare; its transpose G^T aggregates (segment-sums) both the
      messages and the softmax denominators, accumulated in PSUM across
      the window's tiles with the output already transposed. The Wv value
      projection is then applied once per window (linearity), followed by
      a per-head normalization broadcast via a tiny headmask matmul.
  - Softmax skips the segment-max subtraction: dot products here are
    z-scale ~2 (x ~ N(0,1), weights uniform(+-1/sqrt(S))), exp() is safe
    in f32 and the result is mathematically identical.
  - Finale: per 128-node block, unify matmuls accumulate over the 4
    relations in PSUM, ReLU, DMA out. Host concatenates the 8 slices.
"""
import sys

sys.path.insert(0, "/opt/trn_rl_repo")

import numpy as np

N = 50000
R = 4
EMB = 128
H = 4
S = 32
C = 8
NPC = N // C            # 6250 subjects per core
WROWS = 128             # segment rows per window
NWPP = (NPC + WROWS - 1) // WROWS   # windows per relation  (49)
NWIN = R * NWPP         # windows per core (196)
P = 128


def _split_waits(nc, mybir, max_waits=1):
    """This walrus build encodes at most one sync-wait per instruction.
    Hoist excess waits onto NoOp instructions inserted just before."""
    n_split = 0
    for fn in nc.m.functions:
        for block in fn.blocks:
            new_list = []
            for inst in block.instructions:
                si = inst.sync_info
                if si is not None and len(si.on_wait) > max_waits:
                    waits = list(si.on_wait)
                    for w in waits[:-max_waits]:
                        nop = mybir.InstNoOp(
                            name=nc.get_next_instruction_name(),
                            text_hint="waitsplit",
                        )
                        nop.engine = inst.engine
                        nop.sync_info = mybir.SyncInfo(on_wait=[w], on_update=[])
                        new_list.append(nop)
                        n_split += 1
                    inst.sync_info = mybir.SyncInfo(
                        on_wait=waits[-max_waits:], on_update=list(si.on_update)
                    )
                new_list.append(inst)
            block.instructions[:] = new_list
    return n_split


def build_program(n, r, npc, nwpp, tpw, loop_iters=1, gather_mode="indirect"):
    """Build the SPMD Bass program (identical for all cores).

    loop_iters > 1 repeats the compute body inside one dispatch
    (benchmarking only). gather_mode="fake" replaces the indirect gather
    with a contiguous load of the same size (benchmarking only)."""
    import concourse.bass as bass
    import concourse.tile as tile
    from concourse import mybir

    f32 = mybir.dt.float32
    i32 = mybir.dt.int32

    nwin = r * nwpp
    nt = nwin * tpw
    xt_cols = nwpp * P

    nc = bass.Bass()
    x_d = nc.dram_tensor("x", [n, EMB], f32, kind="ExternalInput")
    xt_d = nc.dram_tensor("xt", [EMB, xt_cols], f32, kind="ExternalInput")
    wk_d = nc.dram_tensor("wk", [EMB, r, EMB], f32, kind="ExternalInput")
    wqs_d = nc.dram_tensor("wqs", [EMB, r, EMB], f32, kind="ExternalInput")
    wv_d = nc.dram_tensor("wv", [EMB, r, EMB], f32, kind="ExternalInput")
    ut_d = nc.dram_tensor("ut", [EMB, r, EMB], f32, kind="ExternalInput")
    obj_d = nc.dram_tensor("obj", [P, nt], i32, kind="ExternalInput")
    rid_d = nc.dram_tensor("rid", [P, nt], f32, kind="ExternalInput")
    ridrow_d = nc.dram_tensor("ridrow", [nwin, tpw * P], f32,
                              kind="ExternalInput")
    iota_d = nc.dram_tensor("iota", [P, P], f32, kind="ExternalInput")
    iotat_d = nc.dram_tensor("iotat", [P, P], f32, kind="ExternalInput")
    hm_d = nc.dram_tensor("headmask", [H, P], f32, kind="ExternalInput")
    out_d = nc.dram_tensor("out", [npc, EMB], f32, kind="ExternalOutput")

    with tile.TileContext(nc) as tc, \
         tc.tile_pool(name="const", bufs=1) as constp, \
         tc.tile_pool(name="sbw", bufs=3) as sbw, \
         tc.tile_pool(name="sbt", bufs=4) as sbt, \
         tc.tile_pool(name="psB", bufs=3, space="PSUM") as psB, \
         tc.tile_pool(name="psAgg", bufs=2, space="PSUM") as psAgg, \
         tc.tile_pool(name="psEx", bufs=1, space="PSUM") as psEx, \
         tc.tile_pool(name="psW", bufs=2, space="PSUM") as psW:

        xt_t = constp.tile([P, xt_cols], f32)
        nc.sync.dma_start(out=xt_t[:], in_=xt_d[:])
        wk_t = constp.tile([P, r, EMB], f32)
        nc.sync.dma_start(out=wk_t[:], in_=wk_d[:])
        wqs_t = constp.tile([P, r, EMB], f32)
        nc.sync.dma_start(out=wqs_t[:], in_=wqs_d[:])
        wv_t = constp.tile([P, r, EMB], f32)
        nc.sync.dma_start(out=wv_t[:], in_=wv_d[:])
        ut_t = constp.tile([P, r, EMB], f32)
        nc.sync.dma_start(out=ut_t[:], in_=ut_d[:])
        obj_t = constp.tile([P, nt], i32)
        nc.sync.dma_start(out=obj_t[:], in_=obj_d[:])
        rid_t = constp.tile([P, nt], f32)
        nc.sync.dma_start(out=rid_t[:], in_=rid_d[:])
        iota_t = constp.tile([P, P], f32)
        nc.sync.dma_start(out=iota_t[:], in_=iota_d[:])
        iotat_t = constp.tile([P, P], f32)
        nc.sync.dma_start(out=iotat_t[:], in_=iotat_d[:])
        hm_t = constp.tile([H, P], f32)
        nc.sync.dma_start(out=hm_t[:], in_=hm_d[:])
        ones1_t = constp.tile([1, P], f32)
        nc.vector.memset(ones1_t[:], 1.0)
        aggnt = constp.tile([P, nwin, P], f32)

        for _it in range(loop_iters):
            _kernel_body(nc, tc, bass, mybir, r, npc, nwpp, tpw,
                         xt_t, wk_t, wqs_t, wv_t, ut_t, obj_t, rid_t,
                         ridrow_d, iota_t, iotat_t, hm_t, ones1_t, aggnt,
                         x_d, out_d, sbw, sbt, psB, psAgg, psEx, psW,
                         gather_mode)

    _split_waits(nc, mybir)
    return nc


def _kernel_body(nc, tc, bass, mybir, r, npc, nwpp, tpw,
                 xt_t, wk_t, wqs_t, wv_t, ut_t, obj_t, rid_t,
                 ridrow_d, iota_t, iotat_t, hm_t, ones1_t, aggnt,
                 x_d, out_d, sbw, sbt, psB, psAgg, psEx, psW,
                 gather_mode="indirect"):
    f32 = mybir.dt.float32
    Alu = mybir.AluOpType
    Act = mybir.ActivationFunctionType
    Ax = mybir.AxisListType
    nwin = r * nwpp

    def bc3(ap2, sz):
        # [P, k] -> [P, k, sz] free-dim broadcast (stride 0)
        return bass.AP(tensor=ap2.tensor, offset=ap2.offset,
                       ap=[ap2.ap[0], ap2.ap[1], [0, sz]])

    for w in range(nwin):
        pred = w // nwpp
        sb = w % nwpp
        base = sb * P

        # kwinT[(h,s), i] = keys of this window's subjects (transposed)
        kwinT_ps = psW.tile([P, P], f32, space="PSUM", tag="pw")
        nc.tensor.matmul(out=kwinT_ps[:], lhsT=wk_t[:, pred, :],
                         rhs=xt_t[:, base:base + P], start=True, stop=True)
        kwinT = sbw.tile([P, P], f32, tag="kwinT")
        nc.scalar.activation(out=kwinT[:], in_=kwinT_ps[:], func=Act.Copy,
                             scale=1.0)
        # kq[i, (h,j)] = sum_s kwin[i,(h,s)] Wq[r,h,s,j]
        kq_ps = psW.tile([P, P], f32, space="PSUM", tag="pw")
        nc.tensor.matmul(out=kq_ps[:], lhsT=kwinT[:], rhs=wqs_t[:, pred, :],
                         start=True, stop=True)
        kq = sbw.tile([P, P], f32, tag="kq")
        nc.scalar.activation(out=kq[:], in_=kq_ps[:], func=Act.Copy, scale=1.0)
        # row-ids of this window's edges in row layout [1, tpw*128]
        ridrow = sbw.tile([1, tpw * P], f32, tag="ridrow")
        nc.sync.dma_start(out=ridrow[:], in_=ridrow_d[w:w + 1, :])

        aggx_ps = psAgg.tile([P, P], f32, space="PSUM", tag="pagg")
        ext_ps = psEx.tile([H, P], f32, space="PSUM", tag="pex")
        for k in range(tpw):
            t = w * tpw + k
            # gather x[obj] for this tile's 128 edges
            xg = sbt.tile([P, P], f32, tag="xg")
            if gather_mode == "indirect":
                nc.gpsimd.indirect_dma_start(
                    out=xg[:], out_offset=None, in_=x_d[:],
                    in_offset=bass.IndirectOffsetOnAxis(
                        ap=obj_t[:, t:t + 1], axis=0))
            else:  # "fake": contiguous load of same size (benchmark only)
                nc.sync.dma_start(
                    out=xg[:], in_=x_d[(t % 380) * P:(t % 380) * P + P, :])
            # selectors: GT[e, i] and G[i, e] one-hot on rid_rel
            GT = sbt.tile([P, P], f32, tag="GT")
            nc.vector.tensor_tensor(
                out=GT[:], in0=rid_t[:, t:t + 1].to_broadcast([P, P]),
                in1=iota_t[:], op=Alu.is_equal)
            ridb_ps = psB.tile([P, P], f32, space="PSUM", tag="pb")
            nc.tensor.matmul(out=ridb_ps[:], lhsT=ones1_t[:],
                             rhs=ridrow[0:1, k * P:(k + 1) * P],
                             start=True, stop=True)
            G = sbt.tile([P, P], f32, tag="G")
            nc.vector.tensor_tensor(out=G[:], in0=ridb_ps[:], in1=iotat_t[:],
                                    op=Alu.is_equal)
            # kq at each edge's subject
            kqsel_ps = psB.tile([P, P], f32, space="PSUM", tag="pb")
            nc.tensor.matmul(out=kqsel_ps[:], lhsT=G[:], rhs=kq[:],
                             start=True, stop=True)
            # dot per head, exp
            prod = sbt.tile([P, P], f32, tag="prod")
            nc.vector.tensor_tensor(out=prod[:], in0=kqsel_ps[:], in1=xg[:],
                                    op=Alu.mult)
            dot = sbt.tile([P, H], f32, tag="dot")
            nc.vector.tensor_reduce(
                out=dot[:], in_=prod[:].rearrange("p (h s) -> p h s", h=H),
                axis=Ax.X, op=Alu.add)
            msg = sbt.tile([P, P + H], f32, tag="msg")
            nc.scalar.activation(out=msg[:, P:P + H], in_=dot[:],
                                 func=Act.Exp, scale=1.0)
            # exg = ex * x[obj]   (broadcast ex per head)
            nc.vector.tensor_tensor(
                out=msg[:, 0:P].rearrange("p (h s) -> p h s", h=H),
                in0=xg[:].rearrange("p (h s) -> p h s", h=H),
                in1=bc3(msg[:, P:P + H], S),
                op=Alu.mult)
            # transposed segment-sums, accumulated across the window
            nc.tensor.matmul(out=aggx_ps[:], lhsT=msg[:, 0:P], rhs=GT[:],
                             start=(k == 0), stop=(k == tpw - 1))
            nc.tensor.matmul(out=ext_ps[:], lhsT=msg[:, P:P + H], rhs=GT[:],
                             start=(k == 0), stop=(k == tpw - 1))

        # per-head normalizer 1/(segsum+eps), broadcast to 32 partitions/head
        recipT = sbw.tile([H, P], f32, tag="recipT")
        nc.scalar.activation(out=recipT[:], in_=ext_ps[:], func=Act.Copy,
                             bias=1e-30, scale=1.0)
        nc.vector.reciprocal(out=recipT[:], in_=recipT[:])
        recipb_ps = psW.tile([P, P], f32, space="PSUM", tag="pw")
        nc.tensor.matmul(out=recipb_ps[:], lhsT=hm_t[:], rhs=recipT[:],
                         start=True, stop=True)
        recipb = sbw.tile([P, P], f32, tag="recipb")
        nc.scalar.activation(out=recipb[:], in_=recipb_ps[:], func=Act.Copy,
                             scale=1.0)
        # value projection of the aggregated raw-x messages (linearity)
        aggx = sbw.tile([P, P], f32, tag="aggx")
        nc.scalar.activation(out=aggx[:], in_=aggx_ps[:], func=Act.Copy,
                             scale=1.0)
        aggvt_ps = psW.tile([P, P], f32, space="PSUM", tag="pw")
        nc.tensor.matmul(out=aggvt_ps[:], lhsT=wv_t[:, pred, :], rhs=aggx[:],
                         start=True, stop=True)
        nc.vector.tensor_tensor(out=aggnt[:, w, :], in0=aggvt_ps[:],
                                in1=recipb[:], op=Alu.mult)

    # finale: out[n, i] = relu(sum_r aggn[r block] @ unify[r]^T)
    for sb in range(nwpp):
        nrows = min(P, npc - sb * P)
        o_ps = psW.tile([P, P], f32, space="PSUM", tag="pw")
        for pred in range(r):
            nc.tensor.matmul(out=o_ps[:], lhsT=aggnt[:, pred * nwpp + sb, :],
                             rhs=ut_t[:, pred, :],
                             start=(pred == 0), stop=(pred == r - 1))
        o_sb = sbw.tile([P, P], f32, tag="osb")
        nc.scalar.activation(out=o_sb[:], in_=o_ps[:], func=Act.Relu,
                             scale=1.0)
        nc.sync.dma_start(out=out_d[sb * P: sb * P + nrows, :],
                          in_=o_sb[:nrows, :])


def host_prep(x, tokeys, toqueries, tovals, unify, edge_sub, edge_pred,
              edge_obj, n, r, c, npc, nwpp):
    """Shard + pack edges per core; pre-arrange weights. Returns
    (in_maps, tpw)."""
    x = np.ascontiguousarray(np.asarray(x, dtype=np.float32))
    tokeys = np.asarray(tokeys, dtype=np.float32)
    toqueries = np.asarray(toqueries, dtype=np.float32)
    tovals = np.asarray(tovals, dtype=np.float32)
    unify = np.asarray(unify, dtype=np.float32)
    sub = np.asarray(edge_sub).astype(np.int64)
    pred = np.asarray(edge_pred).astype(np.int64)
    obj = np.asarray(edge_obj).astype(np.int64)

    nwin = r * nwpp
    h, s = tokeys.shape[1], tokeys.shape[2]

    def blockdiag(wr, transpose_block):
        # -> [emb(row), r, emb(col)]
        bd = np.zeros((r, EMB, EMB), dtype=np.float32)
        for rr in range(r):
            for hh in range(h):
                blk = wr[rr, hh].T if transpose_block else wr[rr, hh]
                bd[rr, hh * s:(hh + 1) * s, hh * s:(hh + 1) * s] = blk
        return np.ascontiguousarray(bd.transpose(1, 0, 2))

    # kwinT: lhsT[(h,j),(h,s)] = Wk[r,h,s,j]  -> transposed blocks
    wk_host = blockdiag(tokeys, True)
    # kq: rhs[(h,s),(h,j)] = Wq[r,h,s,j]      -> blocks as-is
    wqs_host = blockdiag(toqueries, False)
    # aggVT: lhsT[(h,j),(h,s)] = Wv[r,h,s,j]  -> transposed blocks
    wv_host = blockdiag(tovals, True)
    ut_host = np.ascontiguousarray(unify.transpose(2, 0, 1))  # [j, r, i]
    iota_host = np.ascontiguousarray(
        np.broadcast_to(np.arange(P, dtype=np.float32), (P, P)))
    iotat_host = np.ascontiguousarray(iota_host.T)
    hm_host = np.zeros((h, EMB), dtype=np.float32)
    for hh in range(h):
        hm_host[hh, hh * s:(hh + 1) * s] = 1.0

    core = sub // npc
    subloc = sub - core * npc
    win = pred * nwpp + subloc // WROWS
    ridrel = (subloc % WROWS).astype(np.float32)

    percore = []
    tpw = 1
    for cc in range(c):
        m = core == cc
        wc = win[m]
        order = np.argsort(wc, kind="stable")
        wc = wc[order]
        rr = ridrel[m][order]
        ob = obj[m][order]
        counts = np.bincount(wc, minlength=nwin)
        tpw = max(tpw, int(np.ceil(counts.max() / P)))
        starts = np.zeros(nwin, dtype=np.int64)
        starts[1:] = np.cumsum(counts)[:-1]
        rank = np.arange(len(wc)) - starts[wc]
        percore.append((cc, wc, rr, ob, rank))

    nt = nwin * tpw
    in_maps = []
    for cc, wc, rr, ob, rank in percore:
        slot = wc * (tpw * P) + rank
        obj_arr = np.zeros(nt * P, dtype=np.int32)
        rid_arr = np.full(nt * P, -1.0, dtype=np.float32)
        obj_arr[slot] = ob.astype(np.int32)
        rid_arr[slot] = rr
        obj_host = np.ascontiguousarray(obj_arr.reshape(nt, P).T)
        rid_host = np.ascontiguousarray(rid_arr.reshape(nt, P).T)
        ridrow_host = np.ascontiguousarray(rid_arr.reshape(nwin, tpw * P))
        xt_host = np.zeros((EMB, nwpp * P), dtype=np.float32)
        xt_host[:, :npc] = x[cc * npc:(cc + 1) * npc].T
        in_maps.append({
            "x": x, "xt": xt_host,
            "wk": wk_host, "wqs": wqs_host, "wv": wv_host, "ut": ut_host,
            "obj": obj_host, "rid": rid_host, "ridrow": ridrow_host,
            "iota": iota_host, "iotat": iotat_host, "headmask": hm_host,
        })
    return in_maps, tpw


_CACHE = {}


def _get_program(n, r, npc, nwpp, tpw):
    key = (n, r, npc, nwpp, tpw)
    if key not in _CACHE:
        _CACHE[key] = build_program(n, r, npc, nwpp, tpw)
    return _CACHE[key]


def kernel(x, tokeys, toqueries, tovals, unify, edge_sub, edge_pred, edge_obj):
    from concourse.bass_utils import run_bass_kernel_spmd

    in_maps, tpw = host_prep(x, tokeys, toqueries, tovals, unify,
                             edge_sub, edge_pred, edge_obj,
                             N, R, C, NPC, NWPP)
    nc = _get_program(N, R, NPC, NWPP, tpw)
    res = run_bass_kernel_spmd(nc, in_maps, list(range(C)))
    out = np.concatenate([res.results[c]["out"] for c in range(C)], axis=0)
    return np.ascontiguousarray(out, dtype=np.float32)


# revision 14
# speedup vs baseline: 1.0541x; 1.0541x over previous
"""Relational GAT message-passing kernel for 8 Trainium2 NeuronCores.

Strategy (zero-collective, 1D row partitioning):
  - Edges are sharded by subject-node range: core c owns all edges whose
    edge_sub falls in [c*N/8, (c+1)*N/8). Segment rows (sub + pred*N) for
    those subjects live entirely on that core, so segment softmax stats and
    the scatter-add need no cross-core reduction at all.
  - Within a core, edges are grouped into "windows" = (pred, 128-subject
    block). Each window's edges are padded to TPW tiles of 128 edge slots.
  - The only per-edge DRAM traffic is one indirect-DMA gather of x[obj]
    per 128-edge tile. Everything else is dense matmul/vector work:
      dot[e,h] = sum_j x[obj_e,(h,j)] * kq[sub_e,(h,j)], where
      kq = (x_window @ Wk_bd) @ Wq_bd  is computed once per window, and
      the per-edge selection kq[sub_e] is a one-hot selector matmul.
      The selector G is built on-chip from the edge row-ids with an
      iota compare; its transpose G^T aggregates (segment-sums) both the
      messages and the softmax denominators, accumulated in PSUM across
      the window's tiles with the output already transposed. The Wv value
      projection is then applied once per window (linearity), followed by
      a per-head normalization broadcast via a tiny headmask matmul.
  - Softmax skips the segment-max subtraction: dot products here are
    z-scale ~2 (x ~ N(0,1), weights uniform(+-1/sqrt(S))), exp() is safe
    in f32 and the result is mathematically identical.
  - Finale: per 128-node block, unify matmuls accumulate over the 4
    relations in PSUM, ReLU, DMA out. Host concatenates the 8 slices.
"""
import sys

sys.path.insert(0, "/opt/trn_rl_repo")

import numpy as np

N = 50000
R = 4
EMB = 128
H = 4
S = 32
C = 8
NPC = N // C            # 6250 subjects per core
WROWS = 128             # segment rows per window
NWPP = (NPC + WROWS - 1) // WROWS   # windows per relation  (49)
NWIN = R * NWPP         # windows per core (196)
P = 128


def _split_waits(nc, mybir, max_waits=1):
    """This walrus build encodes at most one sync-wait per instruction.
    Hoist excess waits onto NoOp instructions inserted just before."""
    n_split = 0
    for fn in nc.m.functions:
        for block in fn.blocks:
            new_list = []
            for inst in block.instructions:
                si = inst.sync_info
                if si is not None and len(si.on_wait) > max_waits:
                    waits = list(si.on_wait)
                    for w in waits[:-max_waits]:
                        nop = mybir.InstNoOp(
                            name=nc.get_next_instruction_name(),
                            text_hint="waitsplit",
                        )
                        nop.engine = inst.engine
                        nop.sync_info = mybir.SyncInfo(on_wait=[w], on_update=[])
                        new_list.append(nop)
                        n_split += 1
                    inst.sync_info = mybir.SyncInfo(
                        on_wait=waits[-max_waits:], on_update=list(si.on_update)
                    )
                new_list.append(inst)
            block.instructions[:] = new_list
    return n_split


def build_program(n, r, npc, nwpp, tpw, loop_iters=1, gather_mode="indirect"):
    """Build the SPMD Bass program (identical for all cores).

    loop_iters > 1 repeats the compute body inside one dispatch
    (benchmarking only). gather_mode="fake" replaces the indirect gather
    with a contiguous load of the same size (benchmarking only)."""
    import concourse.bass as bass
    import concourse.tile as tile
    from concourse import mybir

    f32 = mybir.dt.float32
    i32 = mybir.dt.int32

    nwin = r * nwpp
    nt = nwin * tpw
    xt_cols = nwpp * P

    nc = bass.Bass()
    x_d = nc.dram_tensor("x", [n, EMB], f32, kind="ExternalInput")
    xt_d = nc.dram_tensor("xt", [EMB, xt_cols], f32, kind="ExternalInput")
    kqw_d = nc.dram_tensor("kqw", [EMB, r, EMB], f32, kind="ExternalInput")
    uvt_d = nc.dram_tensor("uvt", [EMB, r, EMB], f32, kind="ExternalInput")
    obj_d = nc.dram_tensor("obj", [P, nt], i32, kind="ExternalInput")
    rid_d = nc.dram_tensor("rid", [P, nt], f32, kind="ExternalInput")
    ridrow_d = nc.dram_tensor("ridrow", [nwin, tpw * P], f32,
                              kind="ExternalInput")
    iota_d = nc.dram_tensor("iota", [P, P], f32, kind="ExternalInput")
    iotat_d = nc.dram_tensor("iotat", [P, P], f32, kind="ExternalInput")
    hm_d = nc.dram_tensor("headmask", [H, P], f32, kind="ExternalInput")
    out_d = nc.dram_tensor("out", [npc, EMB], f32, kind="ExternalOutput")

    with tile.TileContext(nc) as tc, \
         tc.tile_pool(name="const", bufs=1) as constp, \
         tc.tile_pool(name="sbw", bufs=3) as sbw, \
         tc.tile_pool(name="sbt", bufs=4) as sbt, \
         tc.tile_pool(name="psB", bufs=3, space="PSUM") as psB, \
         tc.tile_pool(name="psAgg", bufs=2, space="PSUM") as psAgg, \
         tc.tile_pool(name="psEx", bufs=2, space="PSUM") as psEx, \
         tc.tile_pool(name="psW", bufs=1, space="PSUM") as psW:

        xt_t = constp.tile([P, xt_cols], f32)
        nc.sync.dma_start(out=xt_t[:], in_=xt_d[:])
        kqw_t = constp.tile([P, r, EMB], f32)
        nc.sync.dma_start(out=kqw_t[:], in_=kqw_d[:])
        uvt_t = constp.tile([P, r, EMB], f32)
        nc.sync.dma_start(out=uvt_t[:], in_=uvt_d[:])
        obj_t = constp.tile([P, nt], i32)
        nc.sync.dma_start(out=obj_t[:], in_=obj_d[:])
        rid_t = constp.tile([P, nt], f32)
        nc.sync.dma_start(out=rid_t[:], in_=rid_d[:])
        iota_t = constp.tile([P, P], f32)
        nc.sync.dma_start(out=iota_t[:], in_=iota_d[:])
        iotat_t = constp.tile([P, P], f32)
        nc.sync.dma_start(out=iotat_t[:], in_=iotat_d[:])
        hm_t = constp.tile([H, P], f32)
        nc.sync.dma_start(out=hm_t[:], in_=hm_d[:])
        ones1_t = constp.tile([1, P], f32)
        nc.vector.memset(ones1_t[:], 1.0)
        aggnt = constp.tile([P, nwin, P], f32)

        for _it in range(loop_iters):
            _kernel_body(nc, tc, bass, mybir, r, npc, nwpp, tpw,
                         xt_t, kqw_t, uvt_t, obj_t, rid_t,
                         ridrow_d, iota_t, iotat_t, hm_t, ones1_t, aggnt,
                         x_d, out_d, sbw, sbt, psB, psAgg, psEx, psW,
                         gather_mode)

    _split_waits(nc, mybir)
    return nc


def _kernel_body(nc, tc, bass, mybir, r, npc, nwpp, tpw,
                 xt_t, kqw_t, uvt_t, obj_t, rid_t,
                 ridrow_d, iota_t, iotat_t, hm_t, ones1_t, aggnt,
                 x_d, out_d, sbw, sbt, psB, psAgg, psEx, psW,
                 gather_mode="indirect"):
    f32 = mybir.dt.float32
    Alu = mybir.AluOpType
    Act = mybir.ActivationFunctionType
    Ax = mybir.AxisListType
    nwin = r * nwpp

    def bc3(ap2, sz):
        # [P, k] -> [P, k, sz] free-dim broadcast (stride 0)
        return bass.AP(tensor=ap2.tensor, offset=ap2.offset,
                       ap=[ap2.ap[0], ap2.ap[1], [0, sz]])

    for w in range(nwin):
        pred = w // nwpp
        sb = w % nwpp
        base = sb * P

        # kq[i, (h,j)] = sum_j' x[i,(h,j')] KQ_bd[(h,j'),(h,j)]  (Wk,Wq fused)
        kq_ps = psW.tile([P, P], f32, space="PSUM", tag="pw")
        nc.tensor.matmul(out=kq_ps[:], lhsT=xt_t[:, base:base + P],
                         rhs=kqw_t[:, pred, :], start=True, stop=True)
        kq = sbw.tile([P, P], f32, tag="kq")
        nc.scalar.activation(out=kq[:], in_=kq_ps[:], func=Act.Copy, scale=1.0)
        # row-ids of this window's edges in row layout [1, tpw*128]
        ridrow = sbw.tile([1, tpw * P], f32, tag="ridrow")
        nc.sync.dma_start(out=ridrow[:], in_=ridrow_d[w:w + 1, :])

        aggx_ps = psAgg.tile([P, P], f32, space="PSUM", tag="pagg")
        ext_ps = psEx.tile([H, P], f32, space="PSUM", tag="pex")
        for k in range(tpw):
            t = w * tpw + k
            # gather x[obj] for this tile's 128 edges
            xg = sbt.tile([P, P], f32, tag="xg")
            if gather_mode == "indirect":
                nc.gpsimd.indirect_dma_start(
                    out=xg[:], out_offset=None, in_=x_d[:],
                    in_offset=bass.IndirectOffsetOnAxis(
                        ap=obj_t[:, t:t + 1], axis=0))
            else:  # "fake": contiguous load of same size (benchmark only)
                nc.sync.dma_start(
                    out=xg[:], in_=x_d[(t % 380) * P:(t % 380) * P + P, :])
            # selectors: GT[e, i] and G[i, e] one-hot on rid_rel
            GT = sbt.tile([P, P], f32, tag="GT")
            nc.vector.tensor_tensor(
                out=GT[:], in0=rid_t[:, t:t + 1].to_broadcast([P, P]),
                in1=iota_t[:], op=Alu.is_equal)
            ridb_ps = psB.tile([P, P], f32, space="PSUM", tag="pb")
            nc.tensor.matmul(out=ridb_ps[:], lhsT=ones1_t[:],
                             rhs=ridrow[0:1, k * P:(k + 1) * P],
                             start=True, stop=True)
            G = sbt.tile([P, P], f32, tag="G")
            nc.vector.tensor_tensor(out=G[:], in0=ridb_ps[:], in1=iotat_t[:],
                                    op=Alu.is_equal)
            # kq at each edge's subject
            kqsel_ps = psB.tile([P, P], f32, space="PSUM", tag="pb")
            nc.tensor.matmul(out=kqsel_ps[:], lhsT=G[:], rhs=kq[:],
                             start=True, stop=True)
            # dot per head, exp
            prod = sbt.tile([P, P], f32, tag="prod")
            nc.vector.tensor_tensor(out=prod[:], in0=kqsel_ps[:], in1=xg[:],
                                    op=Alu.mult)
            dot = sbt.tile([P, H], f32, tag="dot")
            nc.vector.tensor_reduce(
                out=dot[:], in_=prod[:].rearrange("p (h s) -> p h s", h=H),
                axis=Ax.X, op=Alu.add)
            msg = sbt.tile([P, P + H], f32, tag="msg")
            nc.scalar.activation(out=msg[:, P:P + H], in_=dot[:],
                                 func=Act.Exp, scale=1.0)
            # exg = ex * x[obj]   (broadcast ex per head)
            nc.vector.tensor_tensor(
                out=msg[:, 0:P].rearrange("p (h s) -> p h s", h=H),
                in0=xg[:].rearrange("p (h s) -> p h s", h=H),
                in1=bc3(msg[:, P:P + H], S),
                op=Alu.mult)
            # transposed segment-sums, accumulated across the window
            nc.tensor.matmul(out=aggx_ps[:], lhsT=msg[:, 0:P], rhs=GT[:],
                             start=(k == 0), stop=(k == tpw - 1))
            nc.tensor.matmul(out=ext_ps[:], lhsT=msg[:, P:P + H], rhs=GT[:],
                             start=(k == 0), stop=(k == tpw - 1))

        # per-head normalizer 1/(segsum+eps), broadcast to 32 partitions/head
        recipT = sbw.tile([H, P], f32, tag="recipT")
        nc.scalar.activation(out=recipT[:], in_=ext_ps[:], func=Act.Copy,
                             bias=1e-30, scale=1.0)
        nc.vector.reciprocal(out=recipT[:], in_=recipT[:])
        recipb_ps = psW.tile([P, P], f32, space="PSUM", tag="pw")
        nc.tensor.matmul(out=recipb_ps[:], lhsT=hm_t[:], rhs=recipT[:],
                         start=True, stop=True)
        recipb = sbw.tile([P, P], f32, tag="recipb")
        nc.scalar.activation(out=recipb[:], in_=recipb_ps[:], func=Act.Copy,
                             scale=1.0)
        # normalized raw-x aggregate; Wv is folded into the unify weights
        nc.vector.tensor_tensor(out=aggnt[:, w, :], in0=aggx_ps[:],
                                in1=recipb[:], op=Alu.mult)

    # finale: out[n, i] = relu(sum_r aggn[r block] @ unify[r]^T)
    for sb in range(nwpp):
        nrows = min(P, npc - sb * P)
        o_ps = psW.tile([P, P], f32, space="PSUM", tag="pw")
        for pred in range(r):
            nc.tensor.matmul(out=o_ps[:], lhsT=aggnt[:, pred * nwpp + sb, :],
                             rhs=uvt_t[:, pred, :],
                             start=(pred == 0), stop=(pred == r - 1))
        o_sb = sbw.tile([P, P], f32, tag="osb")
        nc.scalar.activation(out=o_sb[:], in_=o_ps[:], func=Act.Relu,
                             scale=1.0)
        nc.sync.dma_start(out=out_d[sb * P: sb * P + nrows, :],
                          in_=o_sb[:nrows, :])


def host_prep(x, tokeys, toqueries, tovals, unify, edge_sub, edge_pred,
              edge_obj, n, r, c, npc, nwpp):
    """Shard + pack edges per core; pre-arrange weights. Returns
    (in_maps, tpw)."""
    x = np.ascontiguousarray(np.asarray(x, dtype=np.float32))
    tokeys = np.asarray(tokeys, dtype=np.float32)
    toqueries = np.asarray(toqueries, dtype=np.float32)
    tovals = np.asarray(tovals, dtype=np.float32)
    unify = np.asarray(unify, dtype=np.float32)
    sub = np.asarray(edge_sub).astype(np.int64)
    pred = np.asarray(edge_pred).astype(np.int64)
    obj = np.asarray(edge_obj).astype(np.int64)

    nwin = r * nwpp
    h, s = tokeys.shape[1], tokeys.shape[2]

    def blockdiag(wr, transpose_block):
        # -> [emb(row), r, emb(col)]
        bd = np.zeros((r, EMB, EMB), dtype=np.float32)
        for rr in range(r):
            for hh in range(h):
                blk = wr[rr, hh].T if transpose_block else wr[rr, hh]
                bd[rr, hh * s:(hh + 1) * s, hh * s:(hh + 1) * s] = blk
        return np.ascontiguousarray(bd.transpose(1, 0, 2))

    # fused key-query: KQ_r[(h,j'),(h,j)] = sum_s Wk[r,h,s,j'] Wq[r,h,s,j]
    kqw = np.zeros((r, EMB, EMB), dtype=np.float32)
    for rr in range(r):
        for hh in range(h):
            kqw[rr, hh * s:(hh + 1) * s, hh * s:(hh + 1) * s] = \
                tokeys[rr, hh].T @ toqueries[rr, hh]
    kqw_host = np.ascontiguousarray(kqw.transpose(1, 0, 2))
    # fused unify*Wv: UVT[(h,j), r, i] = sum_s unify[r,i,(h,s)] Wv[r,h,s,j]
    uvt = np.zeros((r, EMB, EMB), dtype=np.float32)   # [r, (h,j), i]
    for rr in range(r):
        for hh in range(h):
            uvt[rr, hh * s:(hh + 1) * s, :] = \
                tovals[rr, hh].T @ unify[rr][:, hh * s:(hh + 1) * s].T
    uvt_host = np.ascontiguousarray(uvt.transpose(1, 0, 2))
    iota_host = np.ascontiguousarray(
        np.broadcast_to(np.arange(P, dtype=np.float32), (P, P)))
    iotat_host = np.ascontiguousarray(iota_host.T)
    hm_host = np.zeros((h, EMB), dtype=np.float32)
    for hh in range(h):
        hm_host[hh, hh * s:(hh + 1) * s] = 1.0

    core = sub // npc
    subloc = sub - core * npc
    win = pred * nwpp + subloc // WROWS
    ridrel = (subloc % WROWS).astype(np.float32)

    percore = []
    tpw = 1
    for cc in range(c):
        m = core == cc
        wc = win[m]
        order = np.argsort(wc, kind="stable")
        wc = wc[order]
        rr = ridrel[m][order]
        ob = obj[m][order]
        counts = np.bincount(wc, minlength=nwin)
        tpw = max(tpw, int(np.ceil(counts.max() / P)))
        starts = np.zeros(nwin, dtype=np.int64)
        starts[1:] = np.cumsum(counts)[:-1]
        rank = np.arange(len(wc)) - starts[wc]
        percore.append((cc, wc, rr, ob, rank))

    nt = nwin * tpw
    in_maps = []
    for cc, wc, rr, ob, rank in percore:
        slot = wc * (tpw * P) + rank
        obj_arr = np.zeros(nt * P, dtype=np.int32)
        rid_arr = np.full(nt * P, -1.0, dtype=np.float32)
        obj_arr[slot] = ob.astype(np.int32)
        rid_arr[slot] = rr
        obj_host = np.ascontiguousarray(obj_arr.reshape(nt, P).T)
        rid_host = np.ascontiguousarray(rid_arr.reshape(nt, P).T)
        ridrow_host = np.ascontiguousarray(rid_arr.reshape(nwin, tpw * P))
        xt_host = np.zeros((EMB, nwpp * P), dtype=np.float32)
        xt_host[:, :npc] = x[cc * npc:(cc + 1) * npc].T
        in_maps.append({
            "x": x, "xt": xt_host,
            "kqw": kqw_host, "uvt": uvt_host,
            "obj": obj_host, "rid": rid_host, "ridrow": ridrow_host,
            "iota": iota_host, "iotat": iotat_host, "headmask": hm_host,
        })
    return in_maps, tpw


_CACHE = {}


def _get_program(n, r, npc, nwpp, tpw):
    key = (n, r, npc, nwpp, tpw)
    if key not in _CACHE:
        _CACHE[key] = build_program(n, r, npc, nwpp, tpw)
    return _CACHE[key]


def kernel(x, tokeys, toqueries, tovals, unify, edge_sub, edge_pred, edge_obj):
    from concourse.bass_utils import run_bass_kernel_spmd

    in_maps, tpw = host_prep(x, tokeys, toqueries, tovals, unify,
                             edge_sub, edge_pred, edge_obj,
                             N, R, C, NPC, NWPP)
    nc = _get_program(N, R, NPC, NWPP, tpw)
    res = run_bass_kernel_spmd(nc, in_maps, list(range(C)))
    out = np.concatenate([res.results[c]["out"] for c in range(C)], axis=0)
    return np.ascontiguousarray(out, dtype=np.float32)
